# revision 24
# baseline (speedup 1.0000x reference)
"""Trainium2 Bass kernel for DiscreteTimeS4.

Reference computation (per batch element b):
    x_proj = relu(x @ Wi^T + bi)          [T, P]
    u      = x_proj @ B                   [T, H]
    h_t    = a * h_{t-1} + u_t            (diagonal linear scan over T)
    y      = hs @ C                       [T, P]
    out    = y @ Wo^T + bo                [T, O]

Sharding: data-parallel over the batch axis -- core b handles x_seq[b].
Weights replicated. No cross-device communication.

Device strategy (per core):
  - host pre-transposes x to x^T [D, T] so the contraction dim (D) sits on
    SBUF partitions; host fuses W2 = C @ Wo^T so the last two matmuls
    collapse into one; weights ship as packed [128, *] tensors (one DMA
    each).
  - matmuls run in fp16 (10-bit mantissa, fp32 PSUM accumulation, FWL
    weight loads) -- measured end-to-end l2 error ~5e-4 vs the fp32
    reference; the scan itself runs in exact fp32.
  - pipeline over time chunks (PE software-pipelined: chunk c's output
    matmul runs while chunk c+1's input matmuls execute, so the PE never
    waits on the scan; the last chunks are narrow so the post-scan tail is
    short):
      MM1: XP^T[p,t] = Wi @ x^T          (lhsT = Wi^T, K=512)
      ACT: relu(psum + bi) -> SBUF
      MM2: U^T[h,t] = B^T @ XP^T         (lhsT = B, K=256)
      DVE: tensor_tensor_scan: h = a*h + u along t (fp32 state, carry
           chained across chunks via initial=prev[:, -1:], decay broadcast
           via a zero-stride AP)
      MM3: OUT[t,o] = (HS^T)^T @ W2      (lhsT = HS^T tile -> natural [t,o]
           output layout)
      +bo via DVE tensor_add (first half) / ACT copy + GPSIMD add (second
      half), then DMA out: early chunks on the SWDGE ring, late chunks on
      the SP HWDGE ring (idle once the input prefetch drains).
"""

import numpy as np

try:
    import concourse.bass as bass
except ImportError:  # pragma: no cover
    import sys

    sys.path.insert(0, "/opt/trn_rl_repo")
    import concourse.bass as bass

from contextlib import ExitStack

import concourse.mybir as mybir
import concourse.tile as tile
from concourse import bacc
from concourse.bass import ts
from concourse.bass_utils import run_bass_kernel_spmd

BSZ, T, D, P, H, O = 8, 4096, 512, 256, 256, 512
F32 = mybir.dt.float32
F32R = mybir.dt.float32r
F16 = mybir.dt.float16

KD = D // 128  # 4 k-tiles for MM1
KP = P // 128  # 2
KH = H // 128  # 2

# time-chunk widths; narrow tail chunks shorten the post-scan critical path
WIDTHS = (512, 512, 512, 512, 512, 512, 512, 256, 128, 128)
# per chunk: how many output t-subtiles take the DVE bias-add path
# (remaining go ACT copy + GPSIMD add)
N_DVE_BIAS = 2
# chunks with index >= this store on the SP HWDGE ring instead of SWDGE
SYNC_STORE_FROM = 5

# packed fp32r weight layout (free-dim offsets, fp32 elements)
WI_OFF = 0
B_OFF = WI_OFF + KD * P
W2_OFF = B_OFF + KP * H
WPACK_F = W2_OFF + KH * O

_NC_CACHE = {}


def _bcast_free(ap, n):
    """Broadcast a [128, 1] AP along the free dim via zero stride."""
    return bass.AP(tensor=ap.tensor, offset=ap.offset, ap=[list(ap.ap[0]), [0, n]])


def build_nc(all16=True, widths=WIDTHS, n_dve_bias=N_DVE_BIAS):
    key = (all16, widths, n_dve_bias)
    if key in _NC_CACHE:
        return _NC_CACHE[key]
    wdt = F16 if all16 else F32R  # matmul operand dtype
    nch = len(widths)
    toffs = [sum(widths[:i]) for i in range(nch)]
    assert sum(widths) == T
    CHMAX = max(widths)
    MOmax = CHMAX // 128

    nc = bacc.Bacc("TRN2", target_bir_lowering=False, debug=False)

    xT_d = nc.dram_tensor("xT", [D, T], wdt, kind="ExternalInput")
    if all16:
        wi16_d = nc.dram_tensor("wi16", [128, KD * P], F16, kind="ExternalInput")
        wb16_d = nc.dram_tensor(
            "wb16", [128, KP * H + KH * O], F16, kind="ExternalInput"
        )
    else:
        wpack_d = nc.dram_tensor("wpack", [128, WPACK_F], F32R, kind="ExternalInput")
    mpack_d = nc.dram_tensor("mpack", [128, KP + KH], F32, kind="ExternalInput")
    borowf_d = nc.dram_tensor("borowf", [1, O], F32, kind="ExternalInput")
    out_d = nc.dram_tensor("out", [T, O], F32, kind="ExternalOutput")

    with tile.TileContext(nc) as tc, ExitStack() as ctx:
        wpool = ctx.enter_context(tc.tile_pool(name="weights", bufs=1))
        xpool = ctx.enter_context(tc.tile_pool(name="x", bufs=nch))
        xppool = ctx.enter_context(tc.tile_pool(name="xp", bufs=3))
        hspool = ctx.enter_context(tc.tile_pool(name="hs", bufs=3))
        opool = ctx.enter_context(tc.tile_pool(name="osb", bufs=5))
        psA = ctx.enter_context(tc.tile_pool(name="psA", bufs=2, space="PSUM"))
        psB = ctx.enter_context(tc.tile_pool(name="psB", bufs=2, space="PSUM"))
        psO = ctx.enter_context(tc.tile_pool(name="psO", bufs=4, space="PSUM"))

        xT_v = xT_d.ap().rearrange("(k p) t -> p k t", p=128)
        x_tiles = []

        # ---- startup DMA order matters: everything the first two chunks
        # need first, on the SP ring
        if all16:
            wi16_sb = wpool.tile([128, KD * P], F16)
            nc.sync.dma_start(out=wi16_sb, in_=wi16_d.ap())
        else:
            wpack_sb = wpool.tile([128, WPACK_F], F32R)
            nc.sync.dma_start(out=wpack_sb[:, :B_OFF], in_=wpack_d.ap()[:, :B_OFF])
        x0_sb = xpool.tile([128, KD, CHMAX], wdt, name="x_sb0", tag="x_sb")
        nc.sync.dma_start(out=x0_sb[:, :2, : widths[0]], in_=xT_v[:, :2, : widths[0]])
        nc.sync.dma_start(out=x0_sb[:, 2:, : widths[0]], in_=xT_v[:, 2:, : widths[0]])
        x_tiles.append(x0_sb)

        mpack_sb = wpool.tile([128, KP + KH], F32)
        nc.sync.dma_start(out=mpack_sb, in_=mpack_d.ap())
        bicol_sl = [mpack_sb[:, m : m + 1] for m in range(KP)]
        acol_sl = [mpack_sb[:, KP + m : KP + m + 1] for m in range(KH)]

        if all16:
            wb16_sb = wpool.tile([128, KP * H + KH * O], F16)
            nc.sync.dma_start(out=wb16_sb, in_=wb16_d.ap())
        else:
            nc.sync.dma_start(out=wpack_sb[:, B_OFF:], in_=wpack_d.ap()[:, B_OFF:])

        borep_sb = wpool.tile([128, O], F32)
        nc.sync.dma_start(
            out=borep_sb,
            in_=bass.AP(tensor=borowf_d.ap().tensor, offset=0, ap=[[0, 128], [1, O]]),
        )

        # ---- deep x prefetch for the remaining chunks
        for c in range(1, nch):
            w = widths[c]
            x_sb = xpool.tile([128, KD, CHMAX], wdt, name=f"x_sb{c}", tag="x_sb")
            nc.sync.dma_start(
                out=x_sb[:, :, :w], in_=xT_v[:, :, toffs[c] : toffs[c] + w]
            )
            x_tiles.append(x_sb)

        def wiT_sl(k, m):  # lhsT tile [128, 128] for MM1
            if all16:
                return wi16_sb[:, k * P + m * 128 : k * P + (m + 1) * 128]
            return wpack_sb[
                :, WI_OFF + k * P + m * 128 : WI_OFF + k * P + (m + 1) * 128
            ]

        def bmat_sl(k, m):
            if all16:
                return wb16_sb[:, k * H + m * 128 : k * H + (m + 1) * 128]
            return wpack_sb[:, B_OFF + k * H + m * 128 : B_OFF + k * H + (m + 1) * 128]

        def w2_sl(k):
            if all16:
                return wb16_sb[:, KP * H + k * O : KP * H + (k + 1) * O]
            return wpack_sb[:, W2_OFF + k * O : W2_OFF + (k + 1) * O]

        hs_tiles = [None] * nch

        def mm3_block(c):
            w = widths[c]
            stn = w // 128
            o_sb = opool.tile([128, MOmax, O], F32, name=f"o_sb{c}", tag="o_sb")
            hs_sb = hs_tiles[c]
            out_vc = out_d.ap()[toffs[c] : toffs[c] + w, :].rearrange(
                "(s p) o -> p s o", p=128
            )
            st_eng = nc.gpsimd if c < SYNC_STORE_FROM else nc.sync
            ndb = min(n_dve_bias, stn)
            for st in range(stn):
                ps3 = psO.tile([128, O], F32, tag="ps3", name=f"ps3_{c}_{st}")
                for k in range(KH):
                    nc.tensor.matmul(
                        ps3[:, :],
                        hs_sb[:, k, ts(st, 128)],
                        w2_sl(k),
                        start=(k == 0),
                        stop=(k == KH - 1),
                    )
                if st >= ndb:
                    # ACT copies out of PSUM, idle GPSIMD applies the bias
                    nc.scalar.copy(o_sb[:, st, :], ps3[:, :])
                    nc.gpsimd.tensor_add(
                        o_sb[:, st, :], o_sb[:, st, :], borep_sb[:, :]
                    )
                else:
                    nc.vector.tensor_add(o_sb[:, st, :], ps3[:, :], borep_sb[:, :])
                if st == ndb - 1 and stn > ndb:
                    # first half ready (DVE-biased) -> store immediately
                    st_eng.dma_start(out=out_vc[:, :ndb, :], in_=o_sb[:, :ndb, :])
            if stn > ndb:
                st_eng.dma_start(out=out_vc[:, ndb:stn, :], in_=o_sb[:, ndb:stn, :])
            else:
                st_eng.dma_start(out=out_vc[:, :stn, :], in_=o_sb[:, :stn, :])

        def mm1_block(c):
            # MM1 + relu/bias -> xp tile; returns the xp tile
            w = widths[c]
            x_sb = x_tiles[c]
            xp_sb = xppool.tile([128, KP, CHMAX], wdt, name=f"xp_sb{c}", tag="xp_sb")
            for m in range(KP):
                ps1 = psA.tile([128, CHMAX], F32, tag="ps1", name=f"ps1_{c}_{m}")
                for k in range(KD):
                    nc.tensor.matmul(
                        ps1[:, :w],
                        wiT_sl(k, m),
                        x_sb[:, k, :w],
                        start=(k == 0),
                        stop=(k == KD - 1),
                    )
                nc.scalar.activation(
                    out=xp_sb[:, m, :w],
                    in_=ps1[:, :w],
                    func=mybir.ActivationFunctionType.Relu,
                    bias=bicol_sl[m],
                    scale=1.0,
                )
            return xp_sb

        for c in range(nch):
            w = widths[c]
            xp_sb = mm1_block(c)

            # ---- MM2 + scan -> hs_sb
            hs_sb = hspool.tile([128, KH, CHMAX], wdt, name=f"hs_sb{c}", tag="hs_sb")
            for m in range(KH):
                ps2 = psB.tile([128, CHMAX], F32, tag="ps2", name=f"ps2_{c}_{m}")
                for k in range(KP):
                    nc.tensor.matmul(
                        ps2[:, :w],
                        bmat_sl(k, m),
                        xp_sb[:, k, :w],
                        start=(k == 0),
                        stop=(k == KP - 1),
                    )
                init = (
                    0.0
                    if c == 0
                    else hs_tiles[c - 1][:, m, widths[c - 1] - 1 : widths[c - 1]]
                )
                nc.vector.tensor_tensor_scan(
                    out=hs_sb[:, m, :w],
                    data0=_bcast_free(acol_sl[m], w),
                    data1=ps2[:, :w],
                    initial=init,
                    op0=mybir.AluOpType.mult,
                    op1=mybir.AluOpType.add,
                )
            hs_tiles[c] = hs_sb

            # ---- deferred MM3 of the previous chunk (keeps PE off the scan's
            # critical path)
            if c > 0:
                mm3_block(c - 1)
        mm3_block(nch - 1)

    nc.finalize()
    _NC_CACHE[key] = nc
    return nc


def _pack128(w, kt):  # [kt*128, F] -> [128, kt*F]
    return np.transpose(w.reshape(kt, 128, -1), (1, 0, 2)).reshape(128, -1)


def _prep_shared(a, B, C, Wi, bi, Wo, bo, all16=True):
    w2 = (C.astype(np.float64) @ Wo.astype(np.float64).T).astype(np.float32)
    shared = {
        "mpack": np.ascontiguousarray(
            np.concatenate([bi.reshape(KP, 128).T, a.reshape(KH, 128).T], axis=1)
        ).astype(np.float32),
        "borowf": np.ascontiguousarray(bo[None, :]).astype(np.float32),
    }
    wiT = np.ascontiguousarray(Wi.T)
    if all16:
        shared["wi16"] = np.ascontiguousarray(_pack128(wiT, KD).astype(np.float16))
        shared["wb16"] = np.ascontiguousarray(
            np.concatenate([_pack128(B, KP), _pack128(w2, KH)], axis=1).astype(
                np.float16
            )
        )
    else:
        shared["wpack"] = np.ascontiguousarray(
            np.concatenate(
                [_pack128(wiT, KD), _pack128(B, KP), _pack128(w2, KH)], axis=1
            )
        )
    return shared


ALL16 = True


def kernel(x_seq, a, B, C, Wi, bi, Wo, bo, _collect=None):
    nc = build_nc(all16=ALL16)
    shared = _prep_shared(a, B, C, Wi, bi, Wo, bo, all16=ALL16)
    xt_dtype = np.float16 if ALL16 else np.float32
    in_maps = []
    for b in range(BSZ):
        m = dict(shared)
        m["xT"] = np.ascontiguousarray(x_seq[b].T.astype(xt_dtype))
        in_maps.append(m)
    kwargs = {}
    if _collect is not None:
        kwargs = {k: v for k, v in _collect.items() if k != "res"}
    res = run_bass_kernel_spmd(nc, in_maps, core_ids=list(range(BSZ)), **kwargs)
    if _collect is not None:
        _collect["res"] = res
    out = np.stack([res.results[b]["out"] for b in range(BSZ)], axis=0)
    return out


# revision 25
# speedup vs baseline: 1.0424x; 1.0424x over previous
"""Trainium2 Bass kernel for DiscreteTimeS4.

Reference computation (per batch element b):
    x_proj = relu(x @ Wi^T + bi)          [T, P]
    u      = x_proj @ B                   [T, H]
    h_t    = a * h_{t-1} + u_t            (diagonal linear scan over T)
    y      = hs @ C                       [T, P]
    out    = y @ Wo^T + bo                [T, O]

Sharding: data-parallel over the batch axis -- core b handles x_seq[b].
Weights replicated. No cross-device communication.

Device strategy (per core):
  - host pre-transposes x to x^T [D, T] so the contraction dim (D) sits on
    SBUF partitions; host fuses W2 = C @ Wo^T so the last two matmuls
    collapse into one; weights ship as packed [128, *] tensors (one DMA
    each).
  - matmuls run in fp16 (10-bit mantissa, fp32 PSUM accumulation, FWL
    weight loads) -- measured end-to-end l2 error ~5e-4 vs the fp32
    reference; the scan itself runs in exact fp32.
  - pipeline over time chunks (PE software-pipelined: chunk c's output
    matmul runs while chunk c+1's input matmuls execute, so the PE never
    waits on the scan; the last chunks are narrow so the post-scan tail is
    short):
      MM1: XP^T[p,t] = Wi @ x^T          (lhsT = Wi^T, K=512)
      ACT: relu(psum + bi) -> SBUF
      MM2: U^T[h,t] = B^T @ XP^T         (lhsT = B, K=256)
      DVE: tensor_tensor_scan: h = a*h + u along t (fp32 state, carry
           chained across chunks via initial=prev[:, -1:], decay broadcast
           via a zero-stride AP)
      MM3: OUT[t,o] = (HS^T)^T @ W2      (lhsT = HS^T tile -> natural [t,o]
           output layout)
      +bo via DVE tensor_add (first half) / ACT copy + GPSIMD add (second
      half), then DMA out: early chunks on the SWDGE ring, late chunks on
      the SP HWDGE ring (idle once the input prefetch drains).
"""

import numpy as np

try:
    import concourse.bass as bass
except ImportError:  # pragma: no cover
    import sys

    sys.path.insert(0, "/opt/trn_rl_repo")
    import concourse.bass as bass

from contextlib import ExitStack

import concourse.mybir as mybir
import concourse.tile as tile
from concourse import bacc
from concourse.bass import ts
from concourse.bass_utils import run_bass_kernel_spmd

BSZ, T, D, P, H, O = 8, 4096, 512, 256, 256, 512
F32 = mybir.dt.float32
F32R = mybir.dt.float32r
F16 = mybir.dt.float16

KD = D // 128  # 4 k-tiles for MM1
KP = P // 128  # 2
KH = H // 128  # 2

# time-chunk widths; narrow tail chunks shorten the post-scan critical path
WIDTHS = (512, 512, 512, 512, 512, 512, 512, 256, 128, 128)
# per chunk: how many output t-subtiles take the DVE bias-add path
# (remaining go ACT copy + GPSIMD add)
N_DVE_BIAS = 2
# chunks with index >= this store on the SP HWDGE ring instead of SWDGE
SYNC_STORE_FROM = 5

# packed fp32r weight layout (free-dim offsets, fp32 elements)
WI_OFF = 0
B_OFF = WI_OFF + KD * P
W2_OFF = B_OFF + KP * H
WPACK_F = W2_OFF + KH * O

_NC_CACHE = {}


def _bcast_free(ap, n):
    """Broadcast a [128, 1] AP along the free dim via zero stride."""
    return bass.AP(tensor=ap.tensor, offset=ap.offset, ap=[list(ap.ap[0]), [0, n]])


def build_nc(all16=True, widths=WIDTHS, n_dve_bias=N_DVE_BIAS):
    key = (all16, widths, n_dve_bias)
    if key in _NC_CACHE:
        return _NC_CACHE[key]
    wdt = F16 if all16 else F32R  # matmul operand dtype
    nch = len(widths)
    toffs = [sum(widths[:i]) for i in range(nch)]
    assert sum(widths) == T
    CHMAX = max(widths)
    MOmax = CHMAX // 128

    nc = bacc.Bacc("TRN2", target_bir_lowering=False, debug=False)

    xT_d = nc.dram_tensor("xT", [D, T], wdt, kind="ExternalInput")
    if all16:
        wi16_d = nc.dram_tensor("wi16", [128, KD * P], F16, kind="ExternalInput")
        wb16_d = nc.dram_tensor(
            "wb16", [128, KP * H + KH * O], F16, kind="ExternalInput"
        )
    else:
        wpack_d = nc.dram_tensor("wpack", [128, WPACK_F], F32R, kind="ExternalInput")
    mpack_d = nc.dram_tensor("mpack", [128, KP + KH], F32, kind="ExternalInput")
    borowf_d = nc.dram_tensor("borowf", [1, O], F32, kind="ExternalInput")
    out_d = nc.dram_tensor("out", [T, O], F32, kind="ExternalOutput")

    with tile.TileContext(nc) as tc, ExitStack() as ctx:
        wpool = ctx.enter_context(tc.tile_pool(name="weights", bufs=1))
        xpool = ctx.enter_context(tc.tile_pool(name="x", bufs=nch))
        xppool = ctx.enter_context(tc.tile_pool(name="xp", bufs=3))
        hspool = ctx.enter_context(tc.tile_pool(name="hs", bufs=3))
        opool = ctx.enter_context(tc.tile_pool(name="osb", bufs=5))
        psA = ctx.enter_context(tc.tile_pool(name="psA", bufs=2, space="PSUM"))
        psB = ctx.enter_context(tc.tile_pool(name="psB", bufs=2, space="PSUM"))
        psO = ctx.enter_context(tc.tile_pool(name="psO", bufs=4, space="PSUM"))

        xT_v = xT_d.ap().rearrange("(k p) t -> p k t", p=128)
        x_tiles = []

        # ---- startup DMA order matters: everything the first two chunks
        # need first, on the SP ring
        if all16:
            wi16_sb = wpool.tile([128, KD * P], F16)
            nc.sync.dma_start(out=wi16_sb, in_=wi16_d.ap())
        else:
            wpack_sb = wpool.tile([128, WPACK_F], F32R)
            nc.sync.dma_start(out=wpack_sb[:, :B_OFF], in_=wpack_d.ap()[:, :B_OFF])
        x0_sb = xpool.tile([128, KD, CHMAX], wdt, name="x_sb0", tag="x_sb")
        nc.sync.dma_start(out=x0_sb[:, :2, : widths[0]], in_=xT_v[:, :2, : widths[0]])
        nc.sync.dma_start(out=x0_sb[:, 2:, : widths[0]], in_=xT_v[:, 2:, : widths[0]])
        x_tiles.append(x0_sb)

        mpack_sb = wpool.tile([128, KP + KH], F32)
        nc.sync.dma_start(out=mpack_sb, in_=mpack_d.ap())
        bicol_sl = [mpack_sb[:, m : m + 1] for m in range(KP)]
        acol_sl = [mpack_sb[:, KP + m : KP + m + 1] for m in range(KH)]

        if all16:
            wb16_sb = wpool.tile([128, KP * H + KH * O], F16)
            nc.sync.dma_start(out=wb16_sb, in_=wb16_d.ap())
        else:
            nc.sync.dma_start(out=wpack_sb[:, B_OFF:], in_=wpack_d.ap()[:, B_OFF:])

        borep_sb = wpool.tile([128, O], F32)
        nc.sync.dma_start(
            out=borep_sb,
            in_=bass.AP(tensor=borowf_d.ap().tensor, offset=0, ap=[[0, 128], [1, O]]),
        )

        # ---- deep x prefetch for the remaining chunks
        for c in range(1, nch):
            w = widths[c]
            x_sb = xpool.tile([128, KD, CHMAX], wdt, name=f"x_sb{c}", tag="x_sb")
            nc.sync.dma_start(
                out=x_sb[:, :, :w], in_=xT_v[:, :, toffs[c] : toffs[c] + w]
            )
            x_tiles.append(x_sb)

        def wiT_sl(k, m):  # lhsT tile [128, 128] for MM1
            if all16:
                return wi16_sb[:, k * P + m * 128 : k * P + (m + 1) * 128]
            return wpack_sb[
                :, WI_OFF + k * P + m * 128 : WI_OFF + k * P + (m + 1) * 128
            ]

        def bmat_sl(k, m):
            if all16:
                return wb16_sb[:, k * H + m * 128 : k * H + (m + 1) * 128]
            return wpack_sb[:, B_OFF + k * H + m * 128 : B_OFF + k * H + (m + 1) * 128]

        def w2_sl(k):
            if all16:
                return wb16_sb[:, KP * H + k * O : KP * H + (k + 1) * O]
            return wpack_sb[:, W2_OFF + k * O : W2_OFF + (k + 1) * O]

        hs_tiles = [None] * nch

        def mm3_block(c):
            w = widths[c]
            stn = w // 128
            o_sb = opool.tile([128, MOmax, O], F32, name=f"o_sb{c}", tag="o_sb")
            hs_sb = hs_tiles[c]
            out_vc = out_d.ap()[toffs[c] : toffs[c] + w, :].rearrange(
                "(s p) o -> p s o", p=128
            )
            st_eng = nc.gpsimd if c < SYNC_STORE_FROM else nc.sync
            ndb = min(n_dve_bias, stn)
            for st in range(stn):
                ps3 = psO.tile([128, O], F32, tag="ps3", name=f"ps3_{c}_{st}")
                for k in range(KH):
                    nc.tensor.matmul(
                        ps3[:, :],
                        hs_sb[:, k, ts(st, 128)],
                        w2_sl(k),
                        start=(k == 0),
                        stop=(k == KH - 1),
                    )
                if st >= ndb:
                    # ACT copies out of PSUM, idle GPSIMD applies the bias
                    nc.scalar.copy(o_sb[:, st, :], ps3[:, :])
                    nc.gpsimd.tensor_add(
                        o_sb[:, st, :], o_sb[:, st, :], borep_sb[:, :]
                    )
                else:
                    nc.vector.tensor_add(o_sb[:, st, :], ps3[:, :], borep_sb[:, :])
                if st == ndb - 1 and stn > ndb:
                    # first half ready (DVE-biased) -> store immediately
                    st_eng.dma_start(out=out_vc[:, :ndb, :], in_=o_sb[:, :ndb, :])
            if stn > ndb:
                st_eng.dma_start(out=out_vc[:, ndb:stn, :], in_=o_sb[:, ndb:stn, :])
            else:
                st_eng.dma_start(out=out_vc[:, :stn, :], in_=o_sb[:, :stn, :])

        def mm1_block(c):
            # MM1 + relu/bias -> xp tile; returns the xp tile
            w = widths[c]
            x_sb = x_tiles[c]
            xp_sb = xppool.tile([128, KP, CHMAX], wdt, name=f"xp_sb{c}", tag="xp_sb")
            for m in range(KP):
                ps1 = psA.tile([128, CHMAX], F32, tag="ps1", name=f"ps1_{c}_{m}")
                for k in range(KD):
                    nc.tensor.matmul(
                        ps1[:, :w],
                        wiT_sl(k, m),
                        x_sb[:, k, :w],
                        start=(k == 0),
                        stop=(k == KD - 1),
                    )
                nc.scalar.activation(
                    out=xp_sb[:, m, :w],
                    in_=ps1[:, :w],
                    func=mybir.ActivationFunctionType.Relu,
                    bias=bicol_sl[m],
                    scale=1.0,
                )
            return xp_sb

        for c in range(nch):
            w = widths[c]
            xp_sb = mm1_block(c)

            # ---- MM2 + scan -> hs_sb
            hs_sb = hspool.tile([128, KH, CHMAX], wdt, name=f"hs_sb{c}", tag="hs_sb")
            for m in range(KH):
                ps2 = psB.tile([128, CHMAX], F32, tag="ps2", name=f"ps2_{c}_{m}")
                for k in range(KP):
                    nc.tensor.matmul(
                        ps2[:, :w],
                        bmat_sl(k, m),
                        xp_sb[:, k, :w],
                        start=(k == 0),
                        stop=(k == KP - 1),
                    )
                init = (
                    0.0
                    if c == 0
                    else hs_tiles[c - 1][:, m, widths[c - 1] - 1 : widths[c - 1]]
                )
                nc.vector.tensor_tensor_scan(
                    out=hs_sb[:, m, :w],
                    data0=_bcast_free(acol_sl[m], w),
                    data1=ps2[:, :w],
                    initial=init,
                    op0=mybir.AluOpType.mult,
                    op1=mybir.AluOpType.add,
                )
            hs_tiles[c] = hs_sb

            # ---- deferred MM3 of the previous chunk (keeps PE off the scan's
            # critical path)
            if c > 0:
                mm3_block(c - 1)
        mm3_block(nch - 1)

    nc.finalize()
    _NC_CACHE[key] = nc
    return nc


def _pack128(w, kt):  # [kt*128, F] -> [128, kt*F]
    return np.transpose(w.reshape(kt, 128, -1), (1, 0, 2)).reshape(128, -1)


def _prep_shared(a, B, C, Wi, bi, Wo, bo, all16=True):
    w2 = (C.astype(np.float64) @ Wo.astype(np.float64).T).astype(np.float32)
    shared = {
        "mpack": np.ascontiguousarray(
            np.concatenate([bi.reshape(KP, 128).T, a.reshape(KH, 128).T], axis=1)
        ).astype(np.float32),
        "borowf": np.ascontiguousarray(bo[None, :]).astype(np.float32),
    }
    wiT = np.ascontiguousarray(Wi.T)
    if all16:
        shared["wi16"] = np.ascontiguousarray(_pack128(wiT, KD).astype(np.float16))
        shared["wb16"] = np.ascontiguousarray(
            np.concatenate([_pack128(B, KP), _pack128(w2, KH)], axis=1).astype(
                np.float16
            )
        )
    else:
        shared["wpack"] = np.ascontiguousarray(
            np.concatenate(
                [_pack128(wiT, KD), _pack128(B, KP), _pack128(w2, KH)], axis=1
            )
        )
    return shared


ALL16 = True


def kernel(x_seq, a, B, C, Wi, bi, Wo, bo, _collect=None):
    nc = build_nc(all16=ALL16)
    shared = _prep_shared(a, B, C, Wi, bi, Wo, bo, all16=ALL16)
    xt_dtype = np.float16 if ALL16 else np.float32
    in_maps = []
    for b in range(BSZ):
        m = dict(shared)
        m["xT"] = np.ascontiguousarray(x_seq[b].T.astype(xt_dtype))
        in_maps.append(m)
    kwargs = {}
    if _collect is not None:
        kwargs = {k: v for k, v in _collect.items() if k != "res"}
    try:
        res = run_bass_kernel_spmd(nc, in_maps, core_ids=list(range(BSZ)), **kwargs)
    except Exception:
        # one retry for transient device errors
        res = run_bass_kernel_spmd(nc, in_maps, core_ids=list(range(BSZ)), **kwargs)
    if _collect is not None:
        _collect["res"] = res
    out = np.stack([res.results[b]["out"] for b in range(BSZ)], axis=0)
    return out
